# revision 1
# baseline (speedup 1.0000x reference)
"""Multi-head self-attention Bass kernel for 8 TRN2 NeuronCores.

Problem: B=8, N=1024, C=1024, H=16, D=64, fp32.
  qkv = x @ w_qkv.T ; split to q,k,v per head
  attn = softmax(q k^T / sqrt(D)) ; out = attn @ v ; y = out @ w_proj.T + b_proj

Sharding: data-parallel over batch -- core b computes batch element b end to
end.  No collectives.

Per-core dataflow:
  phase 1: qT/kT slabs [o_part, n] (bf16) and v slabs [n_part, per-head 65
           cols] (bf16; 64 v columns + a ones column so the AV matmul also
           produces the softmax denominator in PSUM row 64).
           QKV/proj matmuls run in float32r (full-rate, ~tf32 precision).
  phase 2: per head-pair (rows 0:64 / 64:128 of a slab): scoresT[m,n] =
           kT^T qT (two K=64 matmuls in disjoint PE row groups run
           concurrently), exp via ACT -> bf16.  ACT is the bottleneck here.
  phase 3: out'[d,n] = v'^T @ exp (K=128 over m, accumulated); row 64 =
           softmax denominator; divide rows 0..63 by it (reciprocal +
           DRAM-bounce partition broadcast + DVE multiply).
  phase 4: y = projT^T @ w_projT + b_proj (float32r).
"""

import os
import sys

sys.path.insert(0, "/opt/trn_rl_repo")

import numpy as np

B, N, C = 8, 1024, 1024
H = 16
D = C // H  # 64
SCALE = D ** -0.5  # 0.125
P = 128
CT = C // P  # 8 contraction tiles of 128
NCH = N // 512  # 2 free-dim chunks of 512

_CACHE = {}

LAST_EXEC_NS = None


def _build():
    import concourse.bacc as bacc
    import concourse.tile as tile
    from concourse import mybir

    fp32 = mybir.dt.float32
    fp32r = mybir.dt.float32r
    bf16 = mybir.dt.bfloat16
    AFT = mybir.ActivationFunctionType

    nc = bacc.Bacc(
        "TRN2",
        target_bir_lowering=False,
        debug=False,
        enable_asserts=False,
        num_devices=8,
    )
    xT = nc.dram_tensor("xT", [C, N], fp32r, kind="ExternalInput")
    wqkvT = nc.dram_tensor("wqkvT", [C, 3 * C], fp32r, kind="ExternalInput")
    wprojT = nc.dram_tensor("wprojT", [C, C], fp32r, kind="ExternalInput")
    bproj = nc.dram_tensor("bproj", [C], fp32, kind="ExternalInput")
    y = nc.dram_tensor("y", [N, C], fp32, kind="ExternalOutput")

    with tile.TileContext(nc) as tc:
        with (
            tc.tile_pool(name="consts", bufs=1) as consts,
            tc.tile_pool(name="xp", bufs=8) as xp,
            tc.tile_pool(name="wq", bufs=16) as wq,
            tc.tile_pool(name="qt", bufs=8) as qtp,
            tc.tile_pool(name="kt", bufs=8) as ktp,
            tc.tile_pool(name="vp", bufs=8) as vp,
            tc.tile_pool(name="ex", bufs=17) as exp_pool,
            tc.tile_pool(name="pj", bufs=1) as pjp,
            tc.tile_pool(name="sm", bufs=2) as small,
            tc.tile_pool(name="bc", bufs=3) as bcp,
            tc.tile_pool(name="tm", bufs=2) as tmpp,
            tc.tile_pool(name="ot", bufs=2) as otp,
            tc.tile_pool(name="dscr", bufs=8, space="DRAM") as dscr,
            tc.tile_pool(name="ps", bufs=2, space="PSUM") as psum,
            tc.tile_pool(name="pav", bufs=4, space="PSUM") as psav,
        ):
            # x^T resident: slab ci holds rows c in [128ci, 128ci+128)
            xts = [xp.tile([P, N], fp32r, name=f"xt{i}", tag="xt") for i in range(CT)]

            def load_x(nch):
                for ci in range(CT):
                    nc.sync.dma_start(
                        xts[ci][:, nch * 512 : (nch + 1) * 512],
                        xT.ap()[
                            ci * P : (ci + 1) * P, nch * 512 : (nch + 1) * 512
                        ],
                    )

            bb = consts.tile([P, C], fp32)

            # ---- phase 1 helpers
            qts = [None] * 8
            kts = [None] * 8

            def load_w(oblk):
                wts = []
                for ci in range(CT):
                    wt = wq.tile([P, 512], fp32r, name="wt", tag="wt")
                    nc.sync.dma_start(
                        wt[:],
                        wqkvT.ap()[
                            ci * P : (ci + 1) * P, oblk * 512 : (oblk + 1) * 512
                        ],
                    )
                    wts.append(wt)
                return wts

            def emit_qk_slab(s, wts, ss):
                slab = (qtp if s < 8 else ktp).tile(
                    [P, N], bf16, name="slab", tag="slab"
                )
                if s < 8:
                    qts[s] = slab
                else:
                    kts[s - 8] = slab
                ps = psum.tile([P, N], fp32)
                for nch in range(NCH):
                    for ci in range(CT):
                        nc.tensor.matmul(
                            ps[:, nch * 512 : (nch + 1) * 512],
                            lhsT=wts[ci][:, ss * P : (ss + 1) * P],
                            rhs=xts[ci][:, nch * 512 : (nch + 1) * 512],
                            start=(ci == 0),
                            stop=(ci == CT - 1),
                        )
                nc.vector.tensor_copy(slab[:], ps[:])

            vslabs = []
            vviews = []

            def emit_v():
                for mi in range(CT):
                    vs = vp.tile([P, H * 65], bf16, name="vs", tag="vs")
                    vv = vs[:].rearrange("p (h e) -> p h e", e=65)
                    nc.gpsimd.memset(vv[:, :, 64:65], 1.0)
                    vslabs.append(vs)
                    vviews.append(vv)
                vwts = []
                for vblk in range(2):  # v o-chunks of 512
                    wts = []
                    for ci in range(CT):
                        wt = wq.tile([P, 512], fp32r, name="wt", tag="wt")
                        nc.sync.dma_start(
                            wt[:],
                            wqkvT.ap()[
                                ci * P : (ci + 1) * P,
                                2048 + vblk * 512 : 2048 + (vblk + 1) * 512,
                            ],
                        )
                        wts.append(wt)
                    vwts.append(wts)
                for mi in range(CT):
                    ps = psum.tile([P, N], fp32)
                    for vblk in range(2):
                        for ci in range(CT):
                            nc.tensor.matmul(
                                ps[:, vblk * 512 : (vblk + 1) * 512],
                                lhsT=xts[ci][:, mi * P : (mi + 1) * P],
                                rhs=vwts[vblk][ci][:],
                                start=(ci == 0),
                                stop=(ci == CT - 1),
                            )
                    nc.vector.tensor_copy(
                        vviews[mi][:, :, 0:64],
                        ps[:].rearrange("p (hh d) -> p hh d", d=64),
                    )

            # proj-input slabs [c-chunk 128, n] fp32r (normalized out^T)
            pjs = [pjp.tile([P, N], fp32r, name=f"pj{i}") for i in range(CT)]

            # ---- phases 2+3 per head pair, software-pipelined by one pair
            def emit_scores_exp(s):
                # heads 2s (rows 0:64) and 2s+1 (rows 64:128) of slab s
                ets = {0: [], 64: []}
                for mi in range(CT):
                    for rowlo in (0, 64):
                        et = exp_pool.tile([P, N], bf16, name="et", tag="et")
                        ets[rowlo].append(et)
                    for nch in range(NCH):
                        for rowlo in (0, 64):
                            ps = psum.tile([P, 512], fp32)
                            nc.tensor.matmul(
                                ps[:],
                                lhsT=kts[s][
                                    rowlo : rowlo + 64, mi * P : (mi + 1) * P
                                ],
                                rhs=qts[s][
                                    rowlo : rowlo + 64,
                                    nch * 512 : (nch + 1) * 512,
                                ],
                                start=True,
                                stop=True,
                            )
                            nc.scalar.activation(
                                ets[rowlo][mi][:, nch * 512 : (nch + 1) * 512],
                                ps[:],
                                AFT.Exp,
                                scale=SCALE,
                            )
                return ets

            def emit_av_div(s, ets):
                avs = {}
                for rowlo in (0, 64):
                    for nch in range(NCH):
                        avs[(rowlo, nch)] = psav.tile(
                            [65, 512], fp32, name="av", tag="av"
                        )
                for mi in range(CT):
                    for rowlo in (0, 64):
                        h = 2 * s + (1 if rowlo else 0)
                        for nch in range(NCH):
                            nc.tensor.matmul(
                                avs[(rowlo, nch)][:],
                                lhsT=vviews[mi][:, h, :],
                                rhs=ets[rowlo][mi][
                                    :, nch * 512 : (nch + 1) * 512
                                ],
                                start=(mi == 0),
                                stop=(mi == CT - 1),
                            )
                for rowlo in (0, 64):
                    for nch in range(NCH):
                        av = avs[(rowlo, nch)]
                        rcp = small.tile([65, 512], fp32)
                        nc.vector.reciprocal_approx_fast(rcp[:], av[:])
                        scr = dscr.tile([1, 512], fp32)
                        nc.sync.dma_start(scr[:], rcp[64:65, :])
                        rb = bcp.tile([64, 512], fp32)
                        nc.gpsimd.dma_start(
                            rb[:], scr[0, :].partition_broadcast(64)
                        )
                        dst = pjs[s][
                            rowlo : rowlo + 64, nch * 512 : (nch + 1) * 512
                        ]
                        if rowlo == 0:
                            nc.vector.tensor_mul(dst, av[0:64, :], rb[:])
                        else:
                            tmp = tmpp.tile([64, 512], fp32r)
                            nc.vector.tensor_mul(tmp[:], av[0:64, :], rb[:])
                            nc.sync.dma_start(dst, tmp[:])

            def emit_scores_exp_mi(s, ets, mi):
                pss = {}
                for rowlo in (0, 64):
                    et = exp_pool.tile([P, N], bf16, name="et", tag="et")
                    ets[rowlo].append(et)
                    pss[rowlo] = psum.tile([P, N], fp32, name="pss", tag="ps")
                for nch in range(NCH):
                    for rowlo in (0, 64):
                        nc.tensor.matmul(
                            pss[rowlo][:, nch * 512 : (nch + 1) * 512],
                            lhsT=kts[s][
                                rowlo : rowlo + 64, mi * P : (mi + 1) * P
                            ],
                            rhs=qts[s][
                                rowlo : rowlo + 64, nch * 512 : (nch + 1) * 512
                            ],
                            start=True,
                            stop=True,
                        )
                for rowlo in (0, 64):
                    nc.scalar.activation(
                        ets[rowlo][mi][:],
                        pss[rowlo][:],
                        AFT.Exp,
                        scale=SCALE,
                    )

            def emit_av_mi(s, ets, avs, mi):
                for rowlo in (0, 64):
                    h = 2 * s + (1 if rowlo else 0)
                    for nch in range(NCH):
                        nc.tensor.matmul(
                            avs[(rowlo, nch)][:],
                            lhsT=vviews[mi][:, h, :],
                            rhs=ets[rowlo][mi][:, nch * 512 : (nch + 1) * 512],
                            start=(mi == 0),
                            stop=(mi == CT - 1),
                        )

            def emit_div(s, avs):
                for rowlo in (0, 64):
                    for nch in range(NCH):
                        av = avs[(rowlo, nch)]
                        rcp = small.tile([65, 512], fp32)
                        nc.vector.reciprocal_approx_fast(rcp[:], av[:])
                        scr = dscr.tile([1, 512], fp32)
                        nc.sync.dma_start(scr[:], rcp[64:65, :])
                        rb = bcp.tile([64, 512], fp32)
                        nc.gpsimd.dma_start(
                            rb[:], scr[0, :].partition_broadcast(64)
                        )
                        dst = pjs[s][
                            rowlo : rowlo + 64, nch * 512 : (nch + 1) * 512
                        ]
                        if rowlo == 0:
                            nc.vector.tensor_mul(dst, av[0:64, :], rb[:])
                        else:
                            tmp = tmpp.tile([64, 512], fp32r)
                            nc.vector.tensor_mul(tmp[:], av[0:64, :], rb[:])
                            nc.sync.dma_start(dst, tmp[:])

            def new_avs():
                return {
                    (rowlo, nch): psav.tile([65, 512], fp32, name="av", tag="av")
                    for rowlo in (0, 64)
                    for nch in range(NCH)
                }

            def emit_pair(s, prev):
                # scores+exp of pair s interleaved (per m-tile) with the AV
                # accumulation of pair prev[0]
                ets = {0: [], 64: []}
                avs = new_avs() if prev is not None else None
                for mi in range(CT):
                    emit_scores_exp_mi(s, ets, mi)
                    if prev is not None:
                        emit_av_mi(prev[0], prev[1], avs, mi)
                if prev is not None:
                    emit_div(prev[0], avs)
                return ets

            tap = os.environ.get("MHSA_KERNEL_DEBUG_TAP", "")
            run_heads = tap in ("", "pj")
            run_proj = tap == ""
            if run_heads:
                # schedule: x(nch0)+w0 interleaved -> slabs 0,8 -> pair-0
                # scores (ACT starts early) -> rest of qk03 -> v -> pairs
                # 1-2 (+AV 0-1) -> late q/k -> pairs 3-7 -> AV 7 -> proj
                load_x(0)
                wts0 = load_w(0)
                load_x(1)
                wts2 = load_w(2)
                emit_qk_slab(0, wts0, 0)
                emit_qk_slab(8, wts2, 0)
                ets0 = emit_scores_exp(0)
                for ss in range(1, 4):
                    emit_qk_slab(ss, wts0, ss)
                    emit_qk_slab(8 + ss, wts2, ss)
                nc.gpsimd.dma_start(bb[:], bproj.ap().partition_broadcast(P))
                emit_v()
                ets1 = emit_pair(1, (0, ets0))
                ets2 = emit_pair(2, (1, ets1))
                wts1 = load_w(1)
                wts3 = load_w(3)
                for ss in range(4):
                    emit_qk_slab(4 + ss, wts1, ss)
                    emit_qk_slab(12 + ss, wts3, ss)
                prev = (2, ets2)
                for s in range(3, CT):
                    ets = emit_pair(s, prev)
                    prev = (s, ets)
                avs = new_avs()
                for mi in range(CT):
                    emit_av_mi(prev[0], prev[1], avs, mi)
                emit_div(prev[0], avs)
            else:
                load_x(0)
                load_x(1)
                nc.gpsimd.dma_start(bb[:], bproj.ap().partition_broadcast(P))
                wts0 = load_w(0)
                wts2 = load_w(2)
                for ss in range(4):
                    emit_qk_slab(ss, wts0, ss)
                    emit_qk_slab(8 + ss, wts2, ss)
                wts1 = load_w(1)
                wts3 = load_w(3)
                for ss in range(4):
                    emit_qk_slab(4 + ss, wts1, ss)
                    emit_qk_slab(12 + ss, wts3, ss)
                emit_v()

            # ---- phase 4: projection + bias
            if run_proj:
                pwts = []
                for och in range(NCH):
                    wts = []
                    for ci in range(CT):
                        wt = wq.tile([P, 512], fp32r, name="wt", tag="wt")
                        nc.sync.dma_start(
                            wt[:],
                            wprojT.ap()[
                                ci * P : (ci + 1) * P,
                                och * 512 : (och + 1) * 512,
                            ],
                        )
                        wts.append(wt)
                    pwts.append(wts)
                for mi in range(CT):
                    ps = psum.tile([P, N], fp32)
                    for och in range(NCH):
                        for ci in range(CT):
                            nc.tensor.matmul(
                                ps[:, och * 512 : (och + 1) * 512],
                                lhsT=pjs[ci][:, mi * P : (mi + 1) * P],
                                rhs=pwts[och][ci][:],
                                start=(ci == 0),
                                stop=(ci == CT - 1),
                            )
                    ot = otp.tile([P, N], fp32)
                    nc.vector.tensor_add(ot[:], ps[:], bb[:])
                    nc.sync.dma_start(
                        y.ap()[mi * P : (mi + 1) * P, :], ot[:]
                    )

            # ---- debug taps
            if tap in ("q", "k"):
                slabs = qts if tap == "q" else kts
                for s in range(8):
                    ct = otp.tile([P, N], fp32, name="dbgt", tag="dbgt")
                    nc.vector.tensor_copy(ct[:], slabs[s][:])
                    nc.sync.dma_start(y.ap()[s * P : (s + 1) * P, :], ct[:])
            elif tap == "v":
                for mi in range(CT):
                    ct = otp.tile([P, N], fp32, name="dbgt", tag="dbgt")
                    nc.vector.tensor_copy(
                        ct[:].rearrange("p (h d) -> p h d", d=64),
                        vviews[mi][:, :, 0:64],
                    )
                    nc.sync.dma_start(y.ap()[mi * P : (mi + 1) * P, :], ct[:])
            elif tap.startswith("exp"):
                hh = int(tap[3:])
                ets = emit_scores_exp(hh // 2)
                for mi in range(CT):
                    ct = otp.tile([P, N], fp32, name="dbgt", tag="dbgt")
                    nc.vector.tensor_copy(ct[:], ets[(hh % 2) * 64][mi][:])
                    nc.sync.dma_start(y.ap()[mi * P : (mi + 1) * P, :], ct[:])
            elif tap == "pj":
                for s in range(CT):
                    ct = otp.tile([P, N], fp32, name="dbgt", tag="dbgt")
                    nc.vector.tensor_copy(ct[:], pjs[s][:])
                    nc.sync.dma_start(y.ap()[s * P : (s + 1) * P, :], ct[:])

    nc.compile()
    return nc


def kernel(x, w_qkv, w_proj, b_proj):
    global LAST_EXEC_NS
    from concourse.bass_utils import run_bass_kernel_spmd

    x = np.asarray(x, dtype=np.float32)
    w_qkv = np.asarray(w_qkv, dtype=np.float32)
    w_proj = np.asarray(w_proj, dtype=np.float32)
    b_proj = np.asarray(b_proj, dtype=np.float32)

    if "nc" not in _CACHE:
        _CACHE["nc"] = _build()
    nc = _CACHE["nc"]

    wqkvT = np.ascontiguousarray(w_qkv.T)
    wprojT = np.ascontiguousarray(w_proj.T)
    in_maps = [
        {
            "xT": np.ascontiguousarray(x[b].T),
            "wqkvT": wqkvT,
            "wprojT": wprojT,
            "bproj": b_proj,
        }
        for b in range(B)
    ]
    res = run_bass_kernel_spmd(nc, in_maps, core_ids=list(range(B)))
    if res.exec_time_ns is not None:
        LAST_EXEC_NS = res.exec_time_ns
    return np.stack([res.results[b]["y"] for b in range(B)], axis=0)



# revision 7
# speedup vs baseline: 1.1893x; 1.1893x over previous
"""Multi-head self-attention Bass kernel for 8 TRN2 NeuronCores (v2).

Problem: B=8, N=1024, C=1024, H=16, D=64, fp32 in/out.
Sharding: data-parallel over batch -- core b computes batch element b.

v2 design (vs baseline 365us):
  - all matmuls bf16 (host casts x/w to bf16; rel err ~5.6e-3 < 2e-2)
  - one continuous PE instruction stream so HAM stays at 2.4 GHz
    (baseline lost ~50us to K=4/8 clock throttle from PE idle gaps)
  - scores psum tiles [128,1024] hold BOTH heads of a slab in the free
    dim ([r0 n-half | r64 n-half]) so a single ACT exp releases both
    row-group matmuls together -> the two K=64 scores MMs run
    concurrently in disjoint PE row groups (tile_position auto-derived
    from base partition 0/64)
  - ACT-gated middle phase: per (pair s, mi) slot emit 4 scores MMs +
    2 exp ACTs + 4 AV MMs (pair s-1) + q/k-slab filler MMs, so exp
    (147us of ACT work) hides under PE work
  - PSUM: scores 2x[128,1024] (4 banks) + AV acc 2x[65,512] (2) +
    fill/proj 2x[128,512] (2) = 8 banks exactly
  - AV: per (rowlo,nch) accumulate [65,512] over 8 m-tiles; row 64 is
    the softmax denominator via a ones column in the v slabs
  - divide: DVE reciprocal + gpsimd partition_broadcast (no DRAM
    bounce) + DVE multiply; odd head needs an SBUF->SBUF DMA to shift
    partitions 0-63 -> 64-127 of the proj-input slab
"""

import os
import sys

sys.path.insert(0, "/opt/trn_rl_repo")

import numpy as np

B, N, C = 8, 1024, 1024
H = 16
D = C // H  # 64
SCALE = D ** -0.5  # 0.125
P = 128
CT = C // P  # 8 contraction tiles of 128

_CACHE = {}

LAST_EXEC_NS = None


def _build():
    import concourse.bacc as bacc
    import concourse.tile as tile
    from concourse import mybir

    fp32 = mybir.dt.float32
    bf16 = mybir.dt.bfloat16
    AFT = mybir.ActivationFunctionType

    nc = bacc.Bacc(
        "TRN2",
        target_bir_lowering=False,
        debug=False,
        enable_asserts=False,
        num_devices=8,
    )
    xT = nc.dram_tensor("xT", [C, N], bf16, kind="ExternalInput")
    wqkvT = nc.dram_tensor("wqkvT", [C, 3 * C], bf16, kind="ExternalInput")
    wprojT = nc.dram_tensor("wprojT", [C, C], bf16, kind="ExternalInput")
    bproj = nc.dram_tensor("bproj", [C], fp32, kind="ExternalInput")
    y = nc.dram_tensor("y", [N, C], fp32, kind="ExternalOutput")

    tap = os.environ.get("MHSA_KERNEL_DEBUG_TAP", "")

    with tile.TileContext(nc) as tc:
        with (
            tc.tile_pool(name="consts", bufs=1) as consts,
            tc.tile_pool(name="xp", bufs=8) as xp,
            tc.tile_pool(name="wq", bufs=32) as wqp,
            tc.tile_pool(name="wp2", bufs=16) as wp2,
            tc.tile_pool(name="qk", bufs=16) as qkp,
            tc.tile_pool(name="vp", bufs=8) as vpp,
            tc.tile_pool(name="et", bufs=28) as etp,
            tc.tile_pool(name="pj", bufs=8) as pjp,
            tc.tile_pool(name="sm", bufs=2) as smp,
            tc.tile_pool(name="rb", bufs=2) as rbp,
            tc.tile_pool(name="tm", bufs=2) as tmp_pool,
            tc.tile_pool(name="ot", bufs=2) as otp,
            tc.tile_pool(name="dscr", bufs=8, space="DRAM") as dscr,
            tc.tile_pool(name="ps_sc", bufs=2, space="PSUM") as ps_sc,
            tc.tile_pool(name="ps_av", bufs=2, space="PSUM") as ps_av,
            tc.tile_pool(name="ps_fl", bufs=2, space="PSUM") as ps_fl,
        ):
            xts = [xp.tile([P, N], bf16, name=f"xt{i}", tag="xt") for i in range(CT)]
            qts = [qkp.tile([P, N], bf16, name=f"qs{i}", tag="qk") for i in range(CT)]
            kts = [qkp.tile([P, N], bf16, name=f"ks{i}", tag="qk") for i in range(CT)]
            vss = [
                vpp.tile([P, H * 65], bf16, name=f"vs{i}", tag="vs") for i in range(CT)
            ]
            vvs = [v[:].rearrange("p (h e) -> p h e", e=65) for v in vss]
            pjs = [pjp.tile([P, N], bf16, name=f"pj{i}", tag="pj") for i in range(CT)]
            bb = consts.tile([P, C], fp32)

            # ---- ACT table warm-up: tiny exp so the ~2.7us table load
            # happens during the prologue DMA, not at the first real exp.
            junk = smp.tile([1, 16], fp32, name="junk", tag="junk")
            junk2 = smp.tile([1, 16], fp32, name="junk2", tag="junk")
            nc.gpsimd.memset(junk[:], 0.0)
            nc.scalar.activation(junk2[:], junk[:], AFT.Exp, scale=1.0)

            # ---- DMAs.  x split across sync+gpsimd queues for faster lead-in.
            for ci in range(CT):
                eng = nc.sync if ci % 2 == 0 else nc.gpsimd
                eng.dma_start(xts[ci][:], xT.ap()[ci * P : (ci + 1) * P, :])
            nc.gpsimd.dma_start(bb[:], bproj.ap().partition_broadcast(P))

            wtiles = {}

            def load_w(oblk, eng):
                ts = []
                for ci in range(CT):
                    wt = wqp.tile([P, 512], bf16, name="wt", tag="wt")
                    eng.dma_start(
                        wt[:],
                        wqkvT.ap()[
                            ci * P : (ci + 1) * P, oblk * 512 : (oblk + 1) * 512
                        ],
                    )
                    ts.append(wt)
                wtiles[oblk] = ts

            # pool rotation order matters: oblk 4,5 buffers get reused by
            # oblk 1,3 after the v slabs finish reading them.
            load_w(4, nc.sync)
            load_w(5, nc.sync)
            load_w(0, nc.sync)
            load_w(2, nc.sync)

            # ---- slab emitters (each "unit" = half a slab: 8 MMs + 1 copy)
            def emit_qk_half(kind, s, half):
                oblk = (0 if kind == "q" else 2) + s // 4
                dst = (qts if kind == "q" else kts)[s]
                ps = ps_fl.tile([P, 512], fp32, name="fl", tag="fl")
                for ci in range(CT):
                    nc.tensor.matmul(
                        ps[:],
                        lhsT=wtiles[oblk][ci][:, (s % 4) * P : (s % 4 + 1) * P],
                        rhs=xts[ci][:, half * 512 : (half + 1) * 512],
                        start=(ci == 0),
                        stop=(ci == CT - 1),
                    )
                nc.vector.tensor_copy(dst[:, half * 512 : (half + 1) * 512], ps[:])

            def emit_v_half(mi, vblk):
                if vblk == 0:
                    nc.gpsimd.memset(vvs[mi][:, :, 64:65], 1.0)
                ps = ps_fl.tile([P, 512], fp32, name="fl", tag="fl")
                for ci in range(CT):
                    nc.tensor.matmul(
                        ps[:],
                        lhsT=xts[ci][:, mi * P : (mi + 1) * P],
                        rhs=wtiles[4 + vblk][ci][:],
                        start=(ci == 0),
                        stop=(ci == CT - 1),
                    )
                nc.vector.tensor_copy(
                    vvs[mi][:, vblk * 8 : (vblk + 1) * 8, 0:64],
                    ps[:].rearrange("p (hh d) -> p hh d", d=64),
                )

            # ---- scores + exp for (pair s, m-tile mi)
            # psum tile layout: cols 0-511 = head 2s (rows 0:64 of slab),
            # cols 512-1023 = head 2s+1 -- both for one n-half.
            eta = [[None] * CT for _ in range(CT)]  # [s][mi] -> n 0:512
            etb = [[None] * CT for _ in range(CT)]  # [s][mi] -> n 512:1024

            def emit_scores(s, mi):
                ta = ps_sc.tile([P, N], fp32, name="sca", tag="sc")
                tb = ps_sc.tile([P, N], fp32, name="scb", tag="sc")
                for t, nch in ((ta, 0), (tb, 1)):
                    for rowlo in (0, 64):
                        nc.tensor.matmul(
                            t[:, (rowlo // 64) * 512 : (rowlo // 64) * 512 + 512],
                            lhsT=kts[s][rowlo : rowlo + 64, mi * P : (mi + 1) * P],
                            rhs=qts[s][
                                rowlo : rowlo + 64, nch * 512 : (nch + 1) * 512
                            ],
                            start=True,
                            stop=True,
                        )
                ea = etp.tile([P, N], bf16, name="eta", tag="et")
                eb = etp.tile([P, N], bf16, name="etb", tag="et")
                nc.scalar.activation(ea[:], ta[:], AFT.Exp, scale=SCALE)
                nc.scalar.activation(eb[:], tb[:], AFT.Exp, scale=SCALE)
                eta[s][mi] = ea
                etb[s][mi] = eb

            # ---- AV for pair sp, staged: nch0 over slots 0-3, nch1 4-7.
            accs = {}

            def emit_av_slot(sp, slot):
                nch = slot // 4
                j = slot % 4
                ets = eta if nch == 0 else etb
                if j == 0:
                    for rowlo in (0, 64):
                        accs[(sp, nch, rowlo)] = ps_av.tile(
                            [65, 512], fp32, name="av", tag="av"
                        )
                for mi_ in (2 * j, 2 * j + 1):
                    for rowlo in (0, 64):
                        nc.tensor.matmul(
                            accs[(sp, nch, rowlo)][:],
                            lhsT=vvs[mi_][:, 2 * sp + rowlo // 64, :],
                            rhs=ets[sp][mi_][
                                :, (rowlo // 64) * 512 : (rowlo // 64) * 512 + 512
                            ],
                            start=(mi_ == 0),
                            stop=(mi_ == 7),
                        )
                if j == 3:
                    emit_div(sp, nch)

            def emit_div(sp, nch):
                for rowlo in (0, 64):
                    acc = accs.pop((sp, nch, rowlo))
                    rcp = smp.tile([65, 512], fp32, name="rcp", tag="rcp")
                    nc.vector.reciprocal_approx_fast(rcp[:], acc[:])
                    scr = dscr.tile([1, 512], fp32, name="scr", tag="scr")
                    nc.sync.dma_start(scr[:], rcp[64:65, :])
                    rb = rbp.tile([64, 512], fp32, name="rb", tag="rb")
                    nc.gpsimd.dma_start(rb[:], scr[0, :].partition_broadcast(64))
                    dst = pjs[sp][
                        rowlo : rowlo + 64, nch * 512 : (nch + 1) * 512
                    ]
                    if rowlo == 0:
                        nc.vector.tensor_mul(dst, acc[0:64, :], rb[:])
                    else:
                        tmp = tmp_pool.tile([64, 512], bf16, name="tmp", tag="tmp")
                        nc.vector.tensor_mul(tmp[:], acc[0:64, :], rb[:])
                        nc.gpsimd.dma_start(dst, tmp[:])

            # ---- prologue PE: v0-3 (chasing x+w DMA), q0,k0, q1,k1
            run_heads = tap in ("", "pj", "et")
            if run_heads:
                for mi in range(4):
                    for vblk in range(2):
                        emit_v_half(mi, vblk)
                for s in (0, 1):
                    for half in range(2):
                        emit_qk_half("q", s, half)
                    for half in range(2):
                        emit_qk_half("k", s, half)

                pwts = []

                def load_wproj():
                    for och in range(2):
                        ts = []
                        for ci in range(CT):
                            wt = wp2.tile([P, 512], bf16, name="pwt", tag="pwt")
                            nc.gpsimd.dma_start(
                                wt[:],
                                wprojT.ap()[
                                    ci * P : (ci + 1) * P,
                                    och * 512 : (och + 1) * 512,
                                ],
                            )
                            ts.append(wt)
                        pwts.append(ts)

                # filler units consumed inside the pair loop
                units = []
                for mi in range(4, 8):
                    for vblk in range(2):
                        units.append(("v", mi, vblk))
                for s in range(2, 8):
                    for half in range(2):
                        units.append(("k", s, half))
                    for half in range(2):
                        units.append(("q", s, half))

                def pop_unit():
                    if units:
                        kind, a, b = units.pop(0)
                        if kind == "v":
                            emit_v_half(a, b)
                        else:
                            emit_qk_half(kind, a, b)

                # ---- main ACT-gated phase
                for s in range(CT):
                    for mi in range(CT):
                        emit_scores(s, mi)
                        if s >= 1:
                            emit_av_slot(s - 1, mi)
                        # filler budget: pair 0 has no AV -> 1 unit/slot;
                        # later pairs 1 unit every other slot
                        if s == 0:
                            pop_unit()
                        elif s <= 6 and mi % 2 == 0:
                            pop_unit()
                        # late weight loads: oblk1/3 reuse the oblk4/5
                        # buffers, so emit only after the last v-slab
                        # filler MMs (end of pair 0) have been emitted.
                        if s == 1 and mi == 0:
                            load_w(1, nc.sync)
                            load_w(3, nc.sync)
                            load_wproj()
                while units:
                    pop_unit()

                # ---- tail: AV(7) + divides interleaved with projection
                def emit_proj(mi):
                    for och in range(2):
                        ps = ps_fl.tile([P, 512], fp32, name="fl", tag="fl")
                        for ci in range(CT):
                            nc.tensor.matmul(
                                ps[:],
                                lhsT=pjs[ci][:, mi * P : (mi + 1) * P],
                                rhs=pwts[och][ci][:],
                                start=(ci == 0),
                                stop=(ci == CT - 1),
                            )
                        ot = otp.tile([P, 512], fp32, name="ot", tag="ot")
                        nc.vector.tensor_add(
                            ot[:], ps[:], bb[:, och * 512 : (och + 1) * 512]
                        )
                        nc.sync.dma_start(
                            y.ap()[mi * P : (mi + 1) * P, och * 512 : (och + 1) * 512],
                            ot[:],
                        )

                for slot in range(4):
                    emit_av_slot(7, slot)  # nch0 + its divide at slot 3
                if tap == "":
                    for mi in range(4):
                        emit_proj(mi)
                for slot in range(4, 8):
                    emit_av_slot(7, slot)  # nch1 + divide
                if tap == "":
                    for mi in range(4, 8):
                        emit_proj(mi)
            else:
                # debug taps for q/k/v only: emit all slabs plainly
                for mi in range(CT):
                    for vblk in range(2):
                        emit_v_half(mi, vblk)
                for s in range(CT):
                    for kind in ("q", "k"):
                        if s >= 4 and (0 if kind == "q" else 2) + 1 not in wtiles:
                            pass
                        for half in range(2):
                            if s == 4 and half == 0 and kind == "q":
                                load_w(1, nc.sync)
                                load_w(3, nc.sync)
                            emit_qk_half(kind, s, half)

            # ---- debug taps
            if tap in ("q", "k"):
                slabs = qts if tap == "q" else kts
                for s in range(CT):
                    ct = otp.tile([P, N], fp32, name="dbgt", tag="dbgt")
                    nc.vector.tensor_copy(ct[:], slabs[s][:])
                    nc.sync.dma_start(y.ap()[s * P : (s + 1) * P, :], ct[:])
            elif tap == "v":
                for mi in range(CT):
                    ct = otp.tile([P, N], fp32, name="dbgt", tag="dbgt")
                    nc.vector.tensor_copy(
                        ct[:].rearrange("p (h d) -> p h d", d=64),
                        vvs[mi][:, :, 0:64],
                    )
                    nc.sync.dma_start(y.ap()[mi * P : (mi + 1) * P, :], ct[:])
            elif tap == "et":
                # dump pair 7's eta tiles (bf16 -> fp32)
                for mi in range(CT):
                    ct = otp.tile([P, N], fp32, name="dbgt", tag="dbgt")
                    nc.vector.tensor_copy(ct[:], eta[7][mi][:])
                    nc.sync.dma_start(y.ap()[mi * P : (mi + 1) * P, :], ct[:])
            elif tap == "pj":
                for s in range(CT):
                    ct = otp.tile([P, N], fp32, name="dbgt", tag="dbgt")
                    nc.vector.tensor_copy(ct[:], pjs[s][:])
                    nc.sync.dma_start(y.ap()[s * P : (s + 1) * P, :], ct[:])

    nc.compile()
    return nc


def kernel(x, w_qkv, w_proj, b_proj):
    global LAST_EXEC_NS
    import ml_dtypes
    from concourse.bass_utils import run_bass_kernel_spmd

    bf = ml_dtypes.bfloat16
    x = np.asarray(x, dtype=np.float32)
    w_qkv = np.asarray(w_qkv, dtype=np.float32)
    w_proj = np.asarray(w_proj, dtype=np.float32)
    b_proj = np.asarray(b_proj, dtype=np.float32)

    if "nc" not in _CACHE:
        _CACHE["nc"] = _build()
    nc = _CACHE["nc"]

    wqkvT = np.ascontiguousarray(w_qkv.astype(bf).T)
    wprojT = np.ascontiguousarray(w_proj.astype(bf).T)
    xb = x.astype(bf)
    in_maps = [
        {
            "xT": np.ascontiguousarray(xb[b].T),
            "wqkvT": wqkvT,
            "wprojT": wprojT,
            "bproj": b_proj,
        }
        for b in range(B)
    ]
    res = run_bass_kernel_spmd(nc, in_maps, core_ids=list(range(B)))
    if res.exec_time_ns is not None:
        LAST_EXEC_NS = res.exec_time_ns
    return np.stack([res.results[b]["y"] for b in range(B)], axis=0)


# revision 12
# speedup vs baseline: 1.2224x; 1.0278x over previous
"""Multi-head self-attention Bass kernel for 8 TRN2 NeuronCores (v2).

Problem: B=8, N=1024, C=1024, H=16, D=64, fp32 in/out.
Sharding: data-parallel over batch -- core b computes batch element b.

v2 design (vs baseline 365us):
  - all matmuls bf16 (host casts x/w to bf16; rel err ~5.6e-3 < 2e-2)
  - one continuous PE instruction stream so HAM stays at 2.4 GHz
    (baseline lost ~50us to K=4/8 clock throttle from PE idle gaps)
  - scores psum tiles [128,1024] hold BOTH heads of a slab in the free
    dim ([r0 n-half | r64 n-half]) so a single ACT exp releases both
    row-group matmuls together -> the two K=64 scores MMs run
    concurrently in disjoint PE row groups (tile_position auto-derived
    from base partition 0/64)
  - ACT-gated middle phase: per (pair s, mi) slot emit 4 scores MMs +
    2 exp ACTs + 4 AV MMs (pair s-1) + q/k-slab filler MMs, so exp
    (147us of ACT work) hides under PE work
  - PSUM: scores 2x[128,1024] (4 banks) + AV acc 2x[65,512] (2) +
    fill/proj 2x[128,512] (2) = 8 banks exactly
  - AV: per (rowlo,nch) accumulate [65,512] over 8 m-tiles; row 64 is
    the softmax denominator via a ones column in the v slabs
  - divide: DVE reciprocal + gpsimd partition_broadcast (no DRAM
    bounce) + DVE multiply; odd head needs an SBUF->SBUF DMA to shift
    partitions 0-63 -> 64-127 of the proj-input slab
"""

import os
import sys

sys.path.insert(0, "/opt/trn_rl_repo")

import numpy as np

B, N, C = 8, 1024, 1024
H = 16
D = C // H  # 64
SCALE = D ** -0.5  # 0.125
P = 128
CT = C // P  # 8 contraction tiles of 128

_CACHE = {}

LAST_EXEC_NS = None


def _build():
    import concourse.bacc as bacc
    import concourse.tile as tile
    from concourse import mybir

    fp32 = mybir.dt.float32
    bf16 = mybir.dt.bfloat16
    AFT = mybir.ActivationFunctionType

    nc = bacc.Bacc(
        "TRN2",
        target_bir_lowering=False,
        debug=False,
        enable_asserts=False,
        num_devices=8,
    )
    xT = nc.dram_tensor("xT", [C, N], bf16, kind="ExternalInput")
    wqkvT = nc.dram_tensor("wqkvT", [C, 3 * C], bf16, kind="ExternalInput")
    wprojT = nc.dram_tensor("wprojT", [C, C], bf16, kind="ExternalInput")
    bproj = nc.dram_tensor("bproj", [C], fp32, kind="ExternalInput")
    y = nc.dram_tensor("y", [N, C], fp32, kind="ExternalOutput")

    tap = os.environ.get("MHSA_KERNEL_DEBUG_TAP", "")

    with tile.TileContext(nc) as tc:
        with (
            tc.tile_pool(name="consts", bufs=1) as consts,
            tc.tile_pool(name="xp", bufs=8) as xp,
            tc.tile_pool(name="wq", bufs=32) as wqp,
            tc.tile_pool(name="wp2", bufs=16) as wp2,
            tc.tile_pool(name="qk", bufs=16) as qkp,
            tc.tile_pool(name="vp", bufs=8) as vpp,
            tc.tile_pool(name="et", bufs=28) as etp,
            tc.tile_pool(name="pj", bufs=8) as pjp,
            tc.tile_pool(name="sm", bufs=2) as smp,
            tc.tile_pool(name="rb", bufs=2) as rbp,
            tc.tile_pool(name="tm", bufs=2) as tmp_pool,
            tc.tile_pool(name="ot", bufs=2) as otp,
            tc.tile_pool(name="dscr", bufs=8, space="DRAM") as dscr,
            tc.tile_pool(name="ps_sc", bufs=2, space="PSUM") as ps_sc,
            tc.tile_pool(name="ps_av", bufs=2, space="PSUM") as ps_av,
            tc.tile_pool(name="ps_fl", bufs=2, space="PSUM") as ps_fl,
        ):
            xts = [xp.tile([P, N], bf16, name=f"xt{i}", tag="xt") for i in range(CT)]
            qts = [qkp.tile([P, N], bf16, name=f"qs{i}", tag="qk") for i in range(CT)]
            kts = [qkp.tile([P, N], bf16, name=f"ks{i}", tag="qk") for i in range(CT)]
            vss = [
                vpp.tile([P, H * 65], bf16, name=f"vs{i}", tag="vs") for i in range(CT)
            ]
            vvs = [v[:].rearrange("p (h e) -> p h e", e=65) for v in vss]
            pjs = [pjp.tile([P, N], bf16, name=f"pj{i}", tag="pj") for i in range(CT)]
            bb = consts.tile([P, C], fp32)

            # ---- ACT table warm-up: tiny exp so the ~2.7us table load
            # happens during the prologue DMA, not at the first real exp.
            junk = smp.tile([1, 16], fp32, name="junk", tag="junk")
            junk2 = smp.tile([1, 16], fp32, name="junk2", tag="junk")
            nc.gpsimd.memset(junk[:], 0.0)
            nc.scalar.activation(junk2[:], junk[:], AFT.Exp, scale=1.0)

            # ---- PE HAM warm-up: ~5us of dependency-free junk matmuls at
            # t=0 so the PE clock is at 2.4 GHz (K=8/8) by the time the
            # DMA-fed prologue matmuls start.  Without this the whole
            # prologue runs at the cold 1.2 GHz clock.
            jw = consts.tile([1, 512], bf16, name="jw")
            nc.gpsimd.memset(jw[:], 0.0)
            for _ in range(24):
                jps = ps_fl.tile([1, 512], fp32, name="jfl", tag="fl")
                nc.tensor.matmul(
                    jps[:], lhsT=jw[0:1, 0:1], rhs=jw[0:1, :], start=True, stop=True
                )

            # ---- DMAs.  x split across sync+gpsimd queues for faster lead-in.
            for ci in range(CT):
                eng = nc.sync if ci % 2 == 0 else nc.gpsimd
                eng.dma_start(xts[ci][:], xT.ap()[ci * P : (ci + 1) * P, :])
            nc.gpsimd.dma_start(bb[:], bproj.ap().partition_broadcast(P))

            wtiles = {}

            def load_w(oblk, eng):
                ts = []
                for ci in range(CT):
                    wt = wqp.tile([P, 512], bf16, name="wt", tag="wt")
                    eng.dma_start(
                        wt[:],
                        wqkvT.ap()[
                            ci * P : (ci + 1) * P, oblk * 512 : (oblk + 1) * 512
                        ],
                    )
                    ts.append(wt)
                wtiles[oblk] = ts

            # pool rotation order matters: oblk 4,5 buffers get reused by
            # oblk 1,3 after the v slabs finish reading them.
            load_w(4, nc.sync)
            load_w(5, nc.sync)
            load_w(0, nc.sync)
            load_w(2, nc.sync)

            # ---- slab emitters (each "unit" = half a slab: 8 MMs + 1 copy)
            def emit_qk_half(kind, s, half):
                oblk = (0 if kind == "q" else 2) + s // 4
                dst = (qts if kind == "q" else kts)[s]
                ps = ps_fl.tile([P, 512], fp32, name="fl", tag="fl")
                for ci in range(CT):
                    nc.tensor.matmul(
                        ps[:],
                        lhsT=wtiles[oblk][ci][:, (s % 4) * P : (s % 4 + 1) * P],
                        rhs=xts[ci][:, half * 512 : (half + 1) * 512],
                        start=(ci == 0),
                        stop=(ci == CT - 1),
                    )
                nc.vector.tensor_copy(dst[:, half * 512 : (half + 1) * 512], ps[:])

            def emit_v_half(mi, vblk):
                if vblk == 0:
                    nc.gpsimd.memset(vvs[mi][:, :, 64:65], 1.0)
                ps = ps_fl.tile([P, 512], fp32, name="fl", tag="fl")
                for ci in range(CT):
                    nc.tensor.matmul(
                        ps[:],
                        lhsT=xts[ci][:, mi * P : (mi + 1) * P],
                        rhs=wtiles[4 + vblk][ci][:],
                        start=(ci == 0),
                        stop=(ci == CT - 1),
                    )
                nc.vector.tensor_copy(
                    vvs[mi][:, vblk * 8 : (vblk + 1) * 8, 0:64],
                    ps[:].rearrange("p (hh d) -> p hh d", d=64),
                )

            # ---- scores + exp for (pair s, m-tile mi)
            # psum tile layout: cols 0-511 = head 2s (rows 0:64 of slab),
            # cols 512-1023 = head 2s+1 -- both for one n-half.
            eta = [[None] * CT for _ in range(CT)]  # [s][mi] -> n 0:512
            etb = [[None] * CT for _ in range(CT)]  # [s][mi] -> n 512:1024

            def emit_scores(s, mi):
                ta = ps_sc.tile([P, N], fp32, name="sca", tag="sc")
                tb = ps_sc.tile([P, N], fp32, name="scb", tag="sc")
                for t, nch in ((ta, 0), (tb, 1)):
                    for rowlo in (0, 64):
                        nc.tensor.matmul(
                            t[:, (rowlo // 64) * 512 : (rowlo // 64) * 512 + 512],
                            lhsT=kts[s][rowlo : rowlo + 64, mi * P : (mi + 1) * P],
                            rhs=qts[s][
                                rowlo : rowlo + 64, nch * 512 : (nch + 1) * 512
                            ],
                            start=True,
                            stop=True,
                        )
                ea = etp.tile([P, N], bf16, name="eta", tag="et")
                eb = etp.tile([P, N], bf16, name="etb", tag="et")
                nc.scalar.activation(ea[:], ta[:], AFT.Exp, scale=SCALE)
                nc.scalar.activation(eb[:], tb[:], AFT.Exp, scale=SCALE)
                eta[s][mi] = ea
                etb[s][mi] = eb

            # ---- AV for pair sp, staged: nch0 over slots 0-3, nch1 4-7.
            accs = {}

            def emit_av_slot(sp, slot, acc_pool=None):
                nch = slot // 4
                j = slot % 4
                ets = eta if nch == 0 else etb
                if j == 0:
                    pool = acc_pool if acc_pool is not None else ps_av
                    tg = "sc" if acc_pool is not None else "av"
                    for rowlo in (0, 64):
                        accs[(sp, nch, rowlo)] = pool.tile(
                            [65, 512], fp32, name="av", tag=tg
                        )
                for mi_ in (2 * j, 2 * j + 1):
                    for rowlo in (0, 64):
                        nc.tensor.matmul(
                            accs[(sp, nch, rowlo)][:],
                            lhsT=vvs[mi_][:, 2 * sp + rowlo // 64, :],
                            rhs=ets[sp][mi_][
                                :, (rowlo // 64) * 512 : (rowlo // 64) * 512 + 512
                            ],
                            start=(mi_ == 0),
                            stop=(mi_ == 7),
                        )
                if j == 3:
                    emit_div(sp, nch)

            def emit_div(sp, nch):
                for rowlo in (0, 64):
                    acc = accs.pop((sp, nch, rowlo))
                    rcp = smp.tile([65, 512], fp32, name="rcp", tag="rcp")
                    nc.vector.reciprocal_approx_fast(rcp[:], acc[:])
                    scr = dscr.tile([1, 512], fp32, name="scr", tag="scr")
                    nc.sync.dma_start(scr[:], rcp[64:65, :])
                    rb = rbp.tile([64, 512], fp32, name="rb", tag="rb")
                    nc.gpsimd.dma_start(rb[:], scr[0, :].partition_broadcast(64))
                    dst = pjs[sp][
                        rowlo : rowlo + 64, nch * 512 : (nch + 1) * 512
                    ]
                    if rowlo == 0:
                        nc.vector.tensor_mul(dst, acc[0:64, :], rb[:])
                    else:
                        tmp = tmp_pool.tile([64, 512], bf16, name="tmp", tag="tmp")
                        nc.vector.tensor_mul(tmp[:], acc[0:64, :], rb[:])
                        nc.gpsimd.dma_start(dst, tmp[:])

            # ---- prologue PE: v0-3 (chasing x+w DMA), q0,k0, q1,k1
            run_heads = tap in ("", "pj", "et")
            if run_heads:
                for mi in range(4):
                    for vblk in range(2):
                        emit_v_half(mi, vblk)
                for s in (0, 1):
                    for half in range(2):
                        emit_qk_half("q", s, half)
                    for half in range(2):
                        emit_qk_half("k", s, half)

                pwts = []

                def load_wproj():
                    for och in range(2):
                        ts = []
                        for ci in range(CT):
                            wt = wp2.tile([P, 512], bf16, name="pwt", tag="pwt")
                            nc.gpsimd.dma_start(
                                wt[:],
                                wprojT.ap()[
                                    ci * P : (ci + 1) * P,
                                    och * 512 : (och + 1) * 512,
                                ],
                            )
                            ts.append(wt)
                        pwts.append(ts)

                # filler units consumed inside the pair loop
                units = []
                for mi in range(4, 8):
                    for vblk in range(2):
                        units.append(("v", mi, vblk))
                for s in range(2, 8):
                    for half in range(2):
                        units.append(("k", s, half))
                    for half in range(2):
                        units.append(("q", s, half))

                def pop_unit():
                    if units:
                        kind, a, b = units.pop(0)
                        if kind == "v":
                            emit_v_half(a, b)
                        else:
                            emit_qk_half(kind, a, b)

                # ---- main ACT-gated phase.  AV for pair s-1 lags one slot
                # behind the scores of pair s so the next pair's scores MMs
                # always precede the AV group that waits on the previous
                # pair's final exp (kills the pair-boundary ACT bubble).
                for s in range(CT):
                    for mi in range(CT):
                        emit_scores(s, mi)
                        if s >= 1 and mi >= 1:
                            emit_av_slot(s - 1, mi - 1)
                        elif s >= 2 and mi == 0:
                            emit_av_slot(s - 2, 7)
                        # filler budget: pair 0 has no AV -> 1 unit/slot;
                        # later pairs 1 unit every other slot
                        if s == 0:
                            pop_unit()
                        elif s <= 6 and mi % 2 == 0:
                            pop_unit()
                        # late weight loads: oblk1/3 reuse the oblk4/5
                        # buffers, so emit only after the last v-slab
                        # filler MMs (end of pair 0) have been emitted.
                        if s == 1 and mi == 0:
                            load_w(1, nc.sync)
                            load_w(3, nc.sync)
                            load_wproj()
                while units:
                    pop_unit()

                # ---- tail: AV(7) + divides interleaved with projection
                def emit_proj(mi):
                    for och in range(2):
                        ps = ps_fl.tile([P, 512], fp32, name="fl", tag="fl")
                        for ci in range(CT):
                            nc.tensor.matmul(
                                ps[:],
                                lhsT=pjs[ci][:, mi * P : (mi + 1) * P],
                                rhs=pwts[och][ci][:],
                                start=(ci == 0),
                                stop=(ci == CT - 1),
                            )
                        ot = otp.tile([P, 512], fp32, name="ot", tag="ot")
                        nc.vector.tensor_add(
                            ot[:], ps[:], bb[:, och * 512 : (och + 1) * 512]
                        )
                        nc.sync.dma_start(
                            y.ap()[mi * P : (mi + 1) * P, och * 512 : (och + 1) * 512],
                            ot[:],
                        )

                # pending AV groups from the one-slot shift
                emit_av_slot(6, 7)
                # pair 7: nch0 through the usual ps_av accumulators; nch1
                # borrows freed scores-psum slots so its MMs can run while
                # nch0's divide chain is still reading ps_av (keeps PE busy
                # across the divide latency -> HAM stays warm into proj).
                for slot in range(4):
                    emit_av_slot(7, slot)
                for slot in range(4, 8):
                    emit_av_slot(7, slot, acc_pool=ps_sc)
                if tap == "":
                    for mi in range(CT):
                        emit_proj(mi)
            else:
                # debug taps for q/k/v only: emit all slabs plainly
                for mi in range(CT):
                    for vblk in range(2):
                        emit_v_half(mi, vblk)
                for s in range(CT):
                    for kind in ("q", "k"):
                        if s >= 4 and (0 if kind == "q" else 2) + 1 not in wtiles:
                            pass
                        for half in range(2):
                            if s == 4 and half == 0 and kind == "q":
                                load_w(1, nc.sync)
                                load_w(3, nc.sync)
                            emit_qk_half(kind, s, half)

            # ---- debug taps
            if tap in ("q", "k"):
                slabs = qts if tap == "q" else kts
                for s in range(CT):
                    ct = otp.tile([P, N], fp32, name="dbgt", tag="dbgt")
                    nc.vector.tensor_copy(ct[:], slabs[s][:])
                    nc.sync.dma_start(y.ap()[s * P : (s + 1) * P, :], ct[:])
            elif tap == "v":
                for mi in range(CT):
                    ct = otp.tile([P, N], fp32, name="dbgt", tag="dbgt")
                    nc.vector.tensor_copy(
                        ct[:].rearrange("p (h d) -> p h d", d=64),
                        vvs[mi][:, :, 0:64],
                    )
                    nc.sync.dma_start(y.ap()[mi * P : (mi + 1) * P, :], ct[:])
            elif tap == "et":
                # dump pair 7's eta tiles (bf16 -> fp32)
                for mi in range(CT):
                    ct = otp.tile([P, N], fp32, name="dbgt", tag="dbgt")
                    nc.vector.tensor_copy(ct[:], eta[7][mi][:])
                    nc.sync.dma_start(y.ap()[mi * P : (mi + 1) * P, :], ct[:])
            elif tap == "pj":
                for s in range(CT):
                    ct = otp.tile([P, N], fp32, name="dbgt", tag="dbgt")
                    nc.vector.tensor_copy(ct[:], pjs[s][:])
                    nc.sync.dma_start(y.ap()[s * P : (s + 1) * P, :], ct[:])

    nc.compile()
    return nc


def kernel(x, w_qkv, w_proj, b_proj):
    global LAST_EXEC_NS
    import ml_dtypes
    from concourse.bass_utils import run_bass_kernel_spmd

    bf = ml_dtypes.bfloat16
    x = np.asarray(x, dtype=np.float32)
    w_qkv = np.asarray(w_qkv, dtype=np.float32)
    w_proj = np.asarray(w_proj, dtype=np.float32)
    b_proj = np.asarray(b_proj, dtype=np.float32)

    if "nc" not in _CACHE:
        _CACHE["nc"] = _build()
    nc = _CACHE["nc"]

    wqkvT = np.ascontiguousarray(w_qkv.astype(bf).T)
    wprojT = np.ascontiguousarray(w_proj.astype(bf).T)
    xb = x.astype(bf)
    in_maps = [
        {
            "xT": np.ascontiguousarray(xb[b].T),
            "wqkvT": wqkvT,
            "wprojT": wprojT,
            "bproj": b_proj,
        }
        for b in range(B)
    ]
    res = run_bass_kernel_spmd(nc, in_maps, core_ids=list(range(B)))
    if res.exec_time_ns is not None:
        LAST_EXEC_NS = res.exec_time_ns
    return np.stack([res.results[b]["y"] for b in range(B)], axis=0)
